# revision 99
# baseline (speedup 1.0000x reference)
"""Deformable Conv2d (adaptive, modulated) for Trainium2 — 8-core SPMD Bass kernel.

Strategy (v6)
-------------
Shard (batch, H) into 8 shards: core = b*4 + hchunk, each computes 32 output
rows of one batch element (4096 positions = 2 passes x 2 groups x 2 halves).

Key layout trick: engine SBUF operands must start at partition 0/32/64/96.
Coordinate planes for TWO groups are stacked in one [128, 1024] tile:
  rows 64q+0:9   x-planes of group 2p+q
  rows 64q+32:41 y-planes of group 2p+q
so every coordinate transform is ONE full-tile DVE op covering both groups
(junk rows compute junk, kept finite), and x/y slices sit at legal bases.

The fused 3x3 conv (PE, bf16) writes offsets directly into the coordinate
tile via ACT; b_p rides the ACT bias, and p_0 + p_n ride the conv PSUM
accumulation as K=3 rank-1 matmul terms.

Bilinear corner weights (bf16) are broadcast across the 128 (px, c)
partitions with a PE "selector" matmul into PSUM + one ACT/DVE copy to SBUF
(no DMA replication). Gather: dma_gather (transpose mode) fetches 512B
tokens = 2x2 pixels x 64 channels from a host-built edge-replicated patch
table. One elementwise bf16 multiply (split DVE/Pool) applies mm*gx*gy; the
4-corner bilinear sum and the final 3x3 stride-3 conv collapse into PE
matmuls with K=(px,c)=128 x 18 (n,r)-tile contraction.

Cross-pass tiles are double-buffered and both halves' gathers are issued
ahead of the apply/final phase so Pool/PE/DVE/ACT pipeline across halves.
"""

import numpy as np
import ml_dtypes

# ---- problem constants (hardcoded per contract) ----
B, C, H, W = 2, 64, 128, 128
KS, N, DIL, PAD = 3, 9, 2, 1
Hp = H + 2 * PAD            # 130
EXT = Hp + 2                # 132 (edge-replicated ext rows/cols)
NPIX = EXT * EXT            # 17424
NCORES = 8
HSH = H // 4                # 32 rows per core
NPOS = HSH * W              # 4096 positions per core
NG = 4                      # groups per core
GPOS = NPOS // NG           # 1024 positions per group
NBLK = GPOS // 128          # 8 pos-blocks of 128 per group
NTOK = GPOS * N             # 9216 gather tokens per group (512B each)
M_CONV = 128                # fused conv rows: x 0:9 | y 32:41 | ad 64:73 | m 96:105
PKW = 148                   # packed f32: ident 0:9 | bias 9 | pn2 10 | bsel 11:139 | lconst 139:148

REPEAT = 1                  # bench hook: run the whole per-core program R times
CONV_F32R = False           # conv matmuls in float32r (1.5 cyc/row) vs f32 (4)

_cache = {}


# ======================================================================
# host-side input preparation
# ======================================================================

def _prep_consts(w_p, b_p, w_m, w_ad, w_conv):
    f32 = np.float32
    # fused conv taps: wt[t, c, m], t = dy*3+dx ; m-rows per M_CONV layout
    wt = np.zeros((9, C, M_CONV), f32)
    rep3 = [0, 1, 2, 0, 1, 2, 0, 1, 2]
    for t in range(9):
        dy, dx = t // 3, t % 3
        wt[t, :, 0:9] = w_p[0:9, :, dy, dx].T          # x-offsets
        wt[t, :, 32:41] = w_p[9:18, :, dy, dx].T       # y-offsets
        wt[t, :, 64:73] = w_ad[rep3, :, dy, dx].T      # adaptive (9 rows)
        wt[t, :, 96:105] = w_m[0:9, :, dy, dx].T       # modulation
    wt = np.ascontiguousarray(wt.transpose(1, 0, 2).reshape(C, 9 * M_CONV))
    # hi/lo bf16 split: conv(x, w) = w_hi.x_hi + (w_lo.x_hi + w_hi.x_lo)
    # exactly to ~2^-18 — chain A is a K=64 bf16 matmul on w_hi/x_hi, chain
    # B one K=128 bf16 matmul with lhs [w_lo; w_hi] against rhs [x_hi; x_lo]
    bf = ml_dtypes.bfloat16
    wt_hi = wt.astype(bf)
    wt_lo = (wt - wt_hi.astype(f32)).astype(bf)
    wtA = wt_hi
    wtB = np.concatenate([wt_lo, wt_hi], axis=0)       # [128, 9*M_CONV]

    r = np.array([-1.0, 0.0, 1.0], f32)
    pnx = np.repeat(r, 3)
    pny = np.tile(r, 3)

    # packed f32 consts [128, PKW]
    pk32 = np.zeros((128, PKW), f32)
    pk32[0:9, 0:9] = np.eye(9, dtype=f32)
    # bsel [3, M_CONV] at rows 0:3, cols 11:116
    pk32[0, 11 + 0:11 + 9] = 1.0                       # x-ones
    pk32[1, 11 + 32:11 + 41] = 1.0                     # y-ones
    pk32[2, 11 + 0:11 + 9] = pnx                       # pn values
    pk32[2, 11 + 32:11 + 41] = pny                     # (cols 11:139 = bsel)
    for o in (0, 64):                                  # lconst cols 139:148
        for n in range(9):
            pk32[o + n, 139 + n] = float(EXT)
            pk32[o + 32 + n, 139 + n] = 1.0

    bias128 = np.zeros((128, 1), f32)
    pn2 = np.zeros((128, 1), f32)
    for q in range(2):
        o = 64 * q
        bias128[o:o + 9, 0] = b_p[0:9]
        bias128[o + 32:o + 41, 0] = b_p[9:18]
        pn2[o:o + 9, 0] = 2.0 * pnx
        pn2[o + 32:o + 41, 0] = 2.0 * pny

    # packed bf16 consts [128, 2441]: selmat | w3 | bsel | lconst
    pk16 = np.zeros((128, 9 * 128 + 18 * 64 + M_CONV + 9), f32)
    for n in range(9):
        for o in (0, 64):
            pk16[o + n, n * 128:n * 128 + 64] = 1.0
            pk16[o + 32 + n, n * 128 + 64:(n + 1) * 128] = 1.0
    for t in range(18):
        n = t % 9
        blk = w_conv[:, :, n // 3, n % 3].T  # [c, o]
        pk16[0:64, 1152 + t * 64:1152 + (t + 1) * 64] = blk
        pk16[64:128, 1152 + t * 64:1152 + (t + 1) * 64] = blk
    # bf16 copies of the base-plane selector and index lconst (all values
    # small exact integers, so the bf16 matmuls stay exact)
    pk16[0:3, 2304:2304 + M_CONV] = pk32[0:3, 11:11 + M_CONV]
    pk16[:, 2304 + M_CONV:2304 + M_CONV + 9] = pk32[:, 139:148]
    pk16 = pk16.astype(ml_dtypes.bfloat16)

    return dict(wtA=wtA, wtB=wtB, pk32=pk32, pk16=pk16, bias128=bias128,
                pn2=pn2)


def _prep_table(xb):
    """xb: [C, H, W] f32 -> 2x2-patch table [NPIX, 256] bf16.

    Entry j = (i=j//EXT, jj=j%EXT): cols 0:128 = pixels (i, jj), (i, jj+1);
    cols 128:256 = pixels (i+1, jj), (i+1, jj+1) (all edge-clamped), each
    pixel as 64 channels.
    """
    xp = np.pad(xb, ((0, 0), (PAD, PAD), (PAD, PAD)))          # [C, 130, 130]
    idx = np.clip(np.arange(EXT) - 1, 0, Hp - 1)
    ext = xp[:, idx][:, :, idx]                                # [C, 132, 132]
    flat = np.ascontiguousarray(ext.transpose(1, 2, 0)).reshape(NPIX, C)
    nxt = np.concatenate([flat[1:], flat[-1:]], axis=0)
    pair = np.concatenate([flat, nxt], axis=1)                 # [NPIX, 128]
    pairdn = np.concatenate([pair[EXT:], np.tile(pair[-1:], (EXT, 1))], axis=0)
    return np.concatenate([pair, pairdn], axis=1).astype(ml_dtypes.bfloat16)


def _prep_core_inputs(core, x, consts):
    b, hc = core // 4, core % 4
    h0 = hc * HSH
    # conv input rows h0-1 .. h0+32 (34 rows), zero padded at batch edges
    xs = np.zeros((C, HSH + 2, W), np.float32)
    lo, hi = h0 - 1, h0 + HSH + 1
    slo, shi = max(lo, 0), min(hi, H)
    xs[:, slo - lo:shi - lo, :] = x[b, :, slo:shi, :]
    # stacked hi/lo bf16 split: rows 0:64 = bf16(x), 64:128 = residual
    bf = ml_dtypes.bfloat16
    xs_hi = xs.astype(bf)
    xs_lo = (xs - xs_hi.astype(np.float32)).astype(bf)
    xs = np.concatenate([xs_hi, xs_lo], axis=0)        # [128, 34, W]

    # base-plane rhs rows: [3, NG*2*512]: per chunk (g, k) cols (2g+k)*512:
    # row0 = x-base for that chunk, row1 = y-base, row2 = ones
    c = np.arange(512, dtype=np.float32)
    brhs = np.zeros((3, NG * 2 * 512), np.float32)
    for g in range(NG):
        for k in range(2):
            sl = slice((2 * g + k) * 512, (2 * g + k + 1) * 512)
            brhs[0, sl] = (h0 + g * 8 + k * 4 + 1) + np.floor(c / W)
            brhs[1, sl] = (c % W) + 1.0
            brhs[2, sl] = 1.0

    m = dict(xs=xs, brhs=brhs.astype(bf), xe=_cache[('xe', b)])
    m.update({k: consts[k] for k in ('wtA', 'wtB', 'pk32', 'pk16',
                                     'bias128', 'pn2')})
    return m


# ======================================================================
# bass program
# ======================================================================

def _emit(nc, tc, t):
    import concourse.bass as bass
    import concourse.mybir as mybir
    from concourse.bass import AP

    dt = mybir.dt
    ALU = mybir.AluOpType
    ACTF = mybir.ActivationFunctionType
    f32, bf16, i16, i32 = dt.float32, dt.bfloat16, dt.int16, dt.int32

    XROW = HSH + 2          # 34
    XCW = W + 2             # 130 padded row width in sbuf
    vec, act, pe, gp, snc = (nc.vector, nc.scalar, nc.tensor,
                             nc.gpsimd, nc.sync)

    HTOK = NTOK // 2        # 4608 tokens per half-group gather (512B each)
    GW = 2 * HTOK           # 9216 G columns per half (col = r*4608 + token)
    with tc.tile_pool(name="const", bufs=1) as cpool, \
         tc.tile_pool(name="coord", bufs=1) as kpool, \
         tc.tile_pool(name="work", bufs=2) as wpool, \
         tc.tile_pool(name="gath", bufs=2) as gpool, \
         tc.tile_pool(name="wrb", bufs=2) as rpool, \
         tc.tile_pool(name="psA", bufs=2, space="PSUM") as psA, \
         tc.tile_pool(name="psO", bufs=1, space="PSUM") as psO, \
         tc.tile_pool(name="psS", bufs=2, space="PSUM") as psS, \
         tc.tile_pool(name="psT", bufs=1, space="PSUM") as psT:

        # ---- load constants / inputs to SBUF (conv deps first) ----
        wtbA = cpool.tile([C, 9 * M_CONV], bf16, tag="wtbA")
        snc.dma_start(out=wtbA[:], in_=t['wtA'][:])
        wtbB = cpool.tile([128, 9 * M_CONV], bf16, tag="wtbB")
        snc.dma_start(out=wtbB[:], in_=t['wtB'][:])
        pkb = cpool.tile([128, PKW], f32, tag="pkb")
        snc.dma_start(out=pkb[:], in_=t['pk32'][:])
        biasb = cpool.tile([128, 1], f32, tag="biasb")
        snc.dma_start(out=biasb[:], in_=t['bias128'][:])
        pn2b = cpool.tile([128, 1], f32, tag="pn2b")
        snc.dma_start(out=pn2b[:], in_=t['pn2'][:])
        mbias = cpool.tile([128, 1], f32, tag="mbias")
        gp.memset(mbias[:], -64.5)

        # [128, .]: partitions 0:64 = bf16(x) (hi), 64:128 = residual (lo)
        xsb = cpool.tile([128, XROW * XCW], bf16, tag="xsb")
        gp.memset(xsb[:], 0.0)
        snc.dma_start(
            out=AP(xsb.tensor, xsb[:].offset + 1,
                   [[XROW * XCW, 128], [XCW, 10], [1, W]]),
            in_=t['xs'][:, 0:10, :])
        snc.dma_start(
            out=AP(xsb.tensor, xsb[:].offset + 10 * XCW + 1,
                   [[XROW * XCW, 128], [XCW, XROW - 10], [1, W]]),
            in_=t['xs'][:, 10:XROW, :])
        pk16b = cpool.tile([128, 9 * 128 + 18 * 64 + M_CONV + 9],
                           bf16, tag="pk16b")
        snc.dma_start(out=pk16b[:], in_=t['pk16'][:])

        idb = pkb[0:9, 0:9]
        W3C = 9 * 128

        def tp_ap(pit):
            return pit[0:128, 0:9]

        # coordinate-stack tiles, [128, GPOS] (2 groups per pass)
        def ktile(tag, d=f32, bufs=1):
            return kpool.tile([128, GPOS], d, tag=tag, name=tag, bufs=bufs)

        xe = t['xe']  # dram [NPIX, 256] bf16

        for _rep in range(REPEAT):
            for p in range(2):  # pass = 2 groups
                brhsb = kpool.tile([3, NG * 512], bf16, tag="brhsb",
                                   name="brhsb")
                snc.dma_start(out=brhsb[:],
                              in_=t['brhs'][:, p * NG * 512:(p + 1) * NG * 512])
                V = ktile("V", f32, bufs=2)
                ADS = ktile("ADS", f32, bufs=2)
                SGM = ktile("SGM", bf16, bufs=2)
                WC0 = ktile("WC0", bf16, bufs=2)
                WC1 = ktile("WC1", bf16, bufs=2)
                I32T = ktile("I32T", i32)
                F = ktile("F")
                FRAC = ktile("FRAC")
                QRB = ktile("QRB")
                MASK = ktile("MASK")
                ADM = ktile("ADM", bf16)
                R1 = ktile("R1", bf16, bufs=2)  # integers <= 130, bf16-exact
                GLT = ktile("GLT", bf16, bufs=2)
                GRB = ktile("GRB", bf16, bufs=2)
                T1 = ktile("T1", bf16, bufs=2)
                T2 = ktile("T2", bf16, bufs=2)
                GYL = ktile("GYL", bf16)
                T1S = ktile("T1S", bf16)
                T2S = ktile("T2S", bf16)

                for q in range(2):
                    g = 2 * p + q
                    o = 64 * q
                    for k in range(2):
                        pc = psA.tile([M_CONV, 512], f32, tag="pc")
                        for tap in range(9):
                            dy, dx = tap // 3, tap % 3
                            toff = (g * 8 + k * 4 + dy) * XCW + dx
                            # chain A: w_hi . x_hi (K=64)
                            rhsA = AP(xsb.tensor, xsb[:].offset + toff,
                                      [[XROW * XCW, C], [XCW, 4], [1, W]])
                            pe.matmul(pc[:],
                                      wtbA[:, tap * M_CONV:(tap + 1) * M_CONV],
                                      rhsA, start=(tap == 0), stop=False)
                            # chain B: w_lo . x_hi + w_hi . x_lo (K=128)
                            rhsB = AP(xsb.tensor, xsb[:].offset + toff,
                                      [[XROW * XCW, 128], [XCW, 4], [1, W]])
                            pe.matmul(pc[:],
                                      wtbB[:, tap * M_CONV:(tap + 1) * M_CONV],
                                      rhsB, start=False, stop=False)
                        # base plane p_0 + p_n via K=3 rank-1 terms, bf16
                        # (small exact integers)
                        pe.matmul(pc[:], pk16b[0:3, 2304:2304 + M_CONV],
                                  brhsb[:, (2 * (g - 2 * p) + k) * 512:
                                        (2 * (g - 2 * p) + k + 1) * 512],
                                  start=False, stop=True)
                        sl = slice(k * 512, (k + 1) * 512)
                        # 32/64-row writes fully initialize every partition
                        # of V/ADS/SGM across the two q's (junk rows finite)
                        act.activation(V[o:o + 64, sl], pc[0:64, :],
                                       ACTF.Identity,
                                       bias=biasb[o:o + 64], scale=1.0)
                        act.activation(ADS[o:o + 32, sl], pc[64:96, :],
                                       ACTF.Sigmoid, scale=-1.0)
                        act.activation(ADS[o + 32:o + 64, sl], pc[64:96, :],
                                       ACTF.Sigmoid, scale=-1.0)
                        act.activation(SGM[o:o + 32, sl], pc[96:128, :],
                                       ACTF.Sigmoid, scale=1.0)

                # ---- coordinate math, full-tile [128, GPOS] ----
                vec.scalar_tensor_tensor(V[:], ADS[:], pn2b[:], V[:],
                                         op0=ALU.mult, op1=ALU.add)
                # floor(V) robust to convert rounding: g=int(V); F=g-(g>V)
                vec.tensor_copy(I32T[:], V[:])
                vec.tensor_copy(F[:], I32T[:])
                vec.tensor_tensor(FRAC[:], F[:], V[:], op=ALU.is_gt)
                vec.tensor_tensor(F[:], F[:], FRAC[:], op=ALU.subtract)
                vec.tensor_tensor(FRAC[:], V[:], F[:], op=ALU.subtract)
                vec.tensor_scalar(QRB[:], F[:], 1.0, 0.0, ALU.add, ALU.max)
                # QLT over F in place
                vec.tensor_scalar(F[:], F[:], 0.0, Hp - 1.0, ALU.max, ALU.min)
                QLT = F
                vec.tensor_scalar(R1[:], QRB[:], Hp + 0.0, None, ALU.min)
                vec.tensor_scalar(QRB[:], QRB[:], Hp - 1.0, None, ALU.min)
                act.activation(MASK[:], V[:], ACTF.Abs, bias=mbias[:],
                               scale=1.0)
                vec.tensor_scalar(MASK[:], MASK[:], 63.5, None, ALU.is_gt)
                vec.tensor_tensor(MASK[:], MASK[:], FRAC[:], op=ALU.mult)
                vec.tensor_tensor(V[:], V[:], MASK[:], op=ALU.subtract)
                vec.tensor_scalar(V[:], V[:], 0.0, Hp - 1.0, ALU.max, ALU.min)
                vec.scalar_tensor_tensor(GLT[:], QLT[:], 1.0, V[:],
                                         op0=ALU.add, op1=ALU.subtract)
                vec.scalar_tensor_tensor(GRB[:], V[:], 1.0, QRB[:],
                                         op0=ALU.add, op1=ALU.subtract)
                # modulation mm = m * ad_m = SGM * (4*ADS - 2)
                vec.tensor_scalar(ADM[:], ADS[:], 4.0, -2.0, ALU.mult, ALU.add)
                vec.tensor_tensor(ADM[0:32, :], ADM[0:32, :], SGM[0:32, :],
                                  op=ALU.mult)
                vec.tensor_tensor(ADM[64:96, :], ADM[64:96, :], SGM[64:96, :],
                                  op=ALU.mult)
                vec.tensor_tensor(T1[:], ADM[:], GLT[:], op=ALU.mult)
                vec.tensor_tensor(T2[:], ADM[:], GRB[:], op=ALU.mult)
                # partition shifts so WC tensor_tensor inputs share bases
                # (walrus: TT SBUF inputs must have equal base partitions)
                for oo in (0, 64):
                    snc.dma_start(out=GYL[oo:oo + 32, :],
                                  in_=GLT[oo + 32:oo + 64, :])
                    snc.dma_start(out=T1S[oo + 32:oo + 64, :],
                                  in_=T1[oo:oo + 32, :])
                    snc.dma_start(out=T2S[oo + 32:oo + 64, :],
                                  in_=T2[oo:oo + 32, :])

                for q in range(2):
                    g = 2 * p + q
                    o = 64 * q
                    # corner weights (bf16): rows o+0:9 = gx*gy_lt,
                    # o+32:41 = gx*gy_rb (32-row writes keep junk initialized)
                    vec.tensor_tensor(WC0[o:o + 32, :], T1[o:o + 32, :],
                                      GYL[o:o + 32, :], op=ALU.mult)
                    vec.tensor_tensor(WC0[o + 32:o + 64, :],
                                      T1S[o + 32:o + 64, :],
                                      GRB[o + 32:o + 64, :], op=ALU.mult)
                    vec.tensor_tensor(WC1[o:o + 32, :], T2[o:o + 32, :],
                                      GYL[o:o + 32, :], op=ALU.mult)
                    vec.tensor_tensor(WC1[o + 32:o + 64, :],
                                      T2S[o + 32:o + 64, :],
                                      GRB[o + 32:o + 64, :], op=ALU.mult)

                    # ---- indices: idxf = EXT*R1x + R1y via lconst matmul,
                    # stored permuted (col 128b + s + 8i <- pi col 128b+16s+i)
                    # so the transpose input is a contiguous 128-block and the
                    # gather token order ends up (bl, s, n, i)
                    idxf = wpool.tile([9, GPOS], f32, tag="idxf")
                    for k in range(2):
                        pit = psT.tile([128, 512], f32, tag="pit")
                        pe.matmul(pit[0:9, :],
                                  pk16b[o:o + 64,
                                        2304 + M_CONV:2304 + M_CONV + 9],
                                  R1[o:o + 64, k * 512:(k + 1) * 512],
                                  start=True, stop=True)
                        act.activation(
                            AP(idxf.tensor, idxf[:].offset + k * 512,
                               [[GPOS, 9], [128, 4], [1, 8], [8, 16]]),
                            pit[0:9, :], ACTF.Copy, scale=1.0)
                    idxt = wpool.tile([128, NBLK * 9], i16, tag="idxt")
                    for bl in range(NBLK):
                        pit = psT.tile([128, 512], f32, tag="pit")
                        pe.transpose(tp_ap(pit),
                                     idxf[:, bl * 128:(bl + 1) * 128], idb)
                        vec.tensor_copy(idxt[:, bl * 9:(bl + 1) * 9],
                                        tp_ap(pit))

                    Gs = []
                    for h in range(2):
                        # shuffle [128,(b,n)] -> wrapped [16, (b*8+s)*9+n]
                        idxw = wpool.tile([16, HTOK // 16], i16, tag="idxw")
                        for bl in range(4):
                            snc.dma_start(
                                out=AP(idxw.tensor, idxw[:].offset + bl * 72,
                                       [[HTOK // 16, 16], [9, 8], [1, 9]]),
                                in_=AP(idxt.tensor,
                                       idxt[:].offset + (h * 4 + bl) * 9,
                                       [[NBLK * 9, 128], [1, 9]]))
                        idxr = wpool.tile([128, HTOK // 16], i16, tag="idxr")
                        snc.dma_start(out=idxr[0:16, :], in_=idxw[:])
                        snc.dma_start(out=idxr[16:32, :], in_=idxr[0:16, :])
                        snc.dma_start(out=idxr[32:64, :], in_=idxr[0:32, :])
                        snc.dma_start(out=idxr[64:128, :], in_=idxr[0:64, :])

                        # ---- gather (512B tokens = 2x2 pixels x 64ch) ----
                        G = gpool.tile([128, GW], bf16, tag="G")
                        gp.dma_gather(
                            out_ap=AP(G.tensor, G[:].offset,
                                      [[GW, 128], [HTOK, 2], [1, HTOK]]),
                            in_ap=xe[:],
                            idxs_ap=idxr[:],
                            num_idxs=HTOK,
                            num_idxs_reg=HTOK,
                            elem_size=256,
                            elem_step=256,
                            transpose=True,
                            single_packet=False)
                        Gs.append(G)

                    for h in range(2):
                        G = Gs[h]
                        # ---- weight broadcast (PE) + apply ----
                        WRB = rpool.tile([128, 18 * 512], bf16, tag="WRB")
                        hs = slice(h * 512, (h + 1) * 512)
                        for j in range(9):
                            slab = psS.tile([128, 1024], f32, tag="slab")
                            for e in range(2):
                                tt = 2 * j + e
                                r, n = tt // 9, tt % 9
                                wc = WC0 if r == 0 else WC1
                                pe.matmul(slab[:, e * 512:(e + 1) * 512],
                                          pk16b[o:o + 64,
                                                n * 128:(n + 1) * 128],
                                          wc[o:o + 64, hs],
                                          start=True, stop=True)
                            if j in (3, 7):
                                vec.tensor_copy(WRB[:, j * 1024:(j + 1) * 1024],
                                                slab[:])
                            else:
                                act.activation(WRB[:, j * 1024:(j + 1) * 1024],
                                               slab[:], ACTF.Copy, scale=1.0)
                        for tt in range(18):
                            r, n = tt // 9, tt % 9
                            sap = [[GW, 128], [1152, 4], [144, 8], [1, 16]]
                            off = r * HTOK + n * 16
                            gsl = AP(G.tensor, G[:].offset + off, sap)
                            wap = AP(WRB.tensor, WRB[:].offset + tt * 512,
                                     [[18 * 512, 128], [128, 4], [16, 8],
                                      [1, 16]])
                            eng = gp if (tt % 9) < 4 else vec
                            eng.tensor_tensor(gsl, gsl, wap, op=ALU.mult)

                        # ---- final matmuls ----
                        po = psO.tile([64, 512], f32, tag="po")
                        for tt in range(18):
                            r, n = tt // 9, tt % 9
                            rhs = AP(G.tensor,
                                     G[:].offset + r * HTOK + n * 16,
                                     [[GW, 128], [1152, 4], [144, 8],
                                      [1, 16]])
                            pe.matmul(po[:],
                                      pk16b[:, W3C + tt * 64:W3C + (tt + 1) * 64],
                                      rhs, start=(tt == 0), stop=(tt == 17))
                        oc = wpool.tile([64, 512], f32, tag="oc")
                        act.activation(oc[:], po[:], ACTF.Copy, scale=1.0)
                        snc.dma_start(
                            out=t['outp'][:, g * GPOS + h * 512:
                                          g * GPOS + (h + 1) * 512],
                            in_=oc[:])


def _build():
    import concourse.bacc as bacc
    import concourse.tile as tile
    import concourse.mybir as mybir
    dt = mybir.dt

    nc = bacc.Bacc("TRN2", target_bir_lowering=False, debug=False)
    t = {}
    specs = [
        ('xs', [128, HSH + 2, W], dt.bfloat16),
        ('xe', [NPIX, 256], dt.bfloat16),
        ('wtA', [C, 9 * M_CONV], dt.bfloat16),
        ('wtB', [128, 9 * M_CONV], dt.bfloat16),
        ('pk32', [128, PKW], dt.float32),
        ('pk16', [128, 9 * 128 + 18 * 64 + M_CONV + 9], dt.bfloat16),
        ('brhs', [3, NG * 2 * 512], dt.bfloat16),
        ('bias128', [128, 1], dt.float32),
        ('pn2', [128, 1], dt.float32),
    ]
    for name, shape, d in specs:
        t[name] = nc.dram_tensor(name, shape, d, kind="ExternalInput").ap()
    t['outp'] = nc.dram_tensor('outp', [64, NPOS], dt.float32,
                               kind="ExternalOutput").ap()
    with tile.TileContext(nc) as tc:
        _emit(nc, tc, t)
    nc.compile()
    return nc


def kernel(x, w_p, b_p, w_m, w_ad, w_conv):
    from concourse.bass_utils import run_bass_kernel_spmd

    x = np.asarray(x, np.float32)
    consts = _prep_consts(np.asarray(w_p, np.float32), np.asarray(b_p, np.float32),
                          np.asarray(w_m, np.float32), np.asarray(w_ad, np.float32),
                          np.asarray(w_conv, np.float32))
    for b in range(B):
        _cache[('xe', b)] = _prep_table(x[b])
    if 'nc' not in _cache:
        _cache['nc'] = _build()
    nc = _cache['nc']

    in_maps = [_prep_core_inputs(c, x, consts) for c in range(NCORES)]
    res = run_bass_kernel_spmd(nc, in_maps, list(range(NCORES)))
    _cache['last_results'] = res

    out = np.zeros((B, 64, H, W), np.float32)
    for c in range(NCORES):
        b, hc = c // 4, c % 4
        out[b, :, hc * HSH:(hc + 1) * HSH, :] = \
            res.results[c]['outp'].reshape(64, HSH, W)
    return out

